# revision 16
# baseline (speedup 1.0000x reference)
"""Trainium2 Bass kernel for nn_Autocorrelation (B=16, L=1024, D=512, H=8, dh=64).

Self-contained: kernel(**inputs) -> np.ndarray [16, 1024, 512] float32.

Math restructuring vs a straight FFT port:
- Real-input spectrum f in [1, 512] only. The f=0 (DC) term is dropped: in
  stage 1 it shifts every corr lag by a per-row constant (top-k selection and
  softmax are shift-invariant); in stage 2 it is a per-row constant
  rowsum(v')*sum(s) restored via the final copy's bias column. alpha
  (2 except Nyquist) is folded into the host inverse matrices Ci/Si.
- A time-constant projection bias only feeds f=0, so q/k/v are projected
  without bq; bq reappears only in the stage-2 DC correction. 1/L rides the
  q and v projections (Wl = Wq/L).
- Even/odd folding halves the forward DFTs: with xe/xo[j'] = x[j'] +- x[L-j']
  (j'=1..512, Nyquist row halved in Ce), Re X = xe @ Ce + x[0], Im X = xo @ Se.
  The x[0] columns enter as fused scalar adds inside the pointwise complex
  products (scalar_tensor_tensor with a per-partition column).
- Softmax normalization 1/sum is per-row, and everything downstream of the
  weights is linear in them, so the kernel correlates with the UNNORMALIZED
  masked weights and multiplies 1/sum into the final output copy.
- top-13 threshold via two DVE max8 rounds + match_replace, read straight
  from corr's PSUM accumulator (no SBUF staging of corr).
- One need-ordered DMA stream on the sync queue paces the kernel:
  Q, Ce, Se, K, Ci, Si, V; output is [row=(b,dh), tau] f16, host reorders.
"""

import threading

import numpy as np

L = 1024
D = 512
DH = 64
BLOC = 2          # batches per core
B = 16
H = 8
KTOP = 13
NCORES = 8
F = 512
FC = 4            # 128-chunks in the folded/spectral dim
JC = 8


def _build_nc(cfg=None):
    from contextlib import ExitStack

    import concourse.bass as bass
    import concourse.mybir as mybir
    import concourse.tile as tile
    from concourse import bacc
    from concourse.masks import make_identity

    f32 = mybir.dt.float32
    f16 = mybir.dt.float16
    AF = mybir.ActivationFunctionType
    ALU = mybir.AluOpType

    nc = bacc.Bacc("TRN2", target_bir_lowering=False, debug=False, num_devices=NCORES)

    Qf = nc.declare_dram_parameter("Qf", [BLOC, D, L], f16, isOutput=False)
    Kf = nc.declare_dram_parameter("Kf", [BLOC, D, L], f16, isOutput=False)
    Vf = nc.declare_dram_parameter("Vf", [BLOC, D, L], f16, isOutput=False)
    Wlf = nc.declare_dram_parameter("Wlf", [D, DH], f16, isOutput=False)   # Wq/L
    Wuf = nc.declare_dram_parameter("Wuf", [D, DH], f16, isOutput=False)   # Wq
    Bcf = nc.declare_dram_parameter("Bcf", [128], f32, isOutput=False)     # tile(bq,2)
    Cef = nc.declare_dram_parameter("Cef", [F, F], f16, isOutput=False)    # folded fwd
    Sef = nc.declare_dram_parameter("Sef", [F, F], f16, isOutput=False)
    Cif = nc.declare_dram_parameter("Cif", [F, F], f16, isOutput=False)    # alpha-folded, tau 0..511
    Sif = nc.declare_dram_parameter("Sif", [F, F], f16, isOutput=False)    # [Si tau=1..511 | Ci tau=512]
    outd = nc.declare_dram_parameter("out", [128, L], f16, isOutput=True)

    with tile.TileContext(nc) as tc, ExitStack() as ctx:
        consts = ctx.enter_context(tc.tile_pool(name="consts", bufs=1))
        inp = ctx.enter_context(tc.tile_pool(name="inp", bufs=1))
        pjp = ctx.enter_context(tc.tile_pool(name="pjp", bufs=2))
        rowsp = ctx.enter_context(tc.tile_pool(name="rowsp", bufs=1))
        specp = ctx.enter_context(tc.tile_pool(name="specp", bufs=1))
        rowbig = ctx.enter_context(tc.tile_pool(name="rowbig", bufs=1))
        small = ctx.enter_context(tc.tile_pool(name="small", bufs=1))
        ps_pj = ctx.enter_context(tc.tile_pool(name="ps_pj", bufs=2, space="PSUM"))
        ps_tr = ctx.enter_context(tc.tile_pool(name="ps_tr", bufs=2, space="PSUM"))
        ps_sp = ctx.enter_context(tc.tile_pool(name="ps_sp", bufs=4, space="PSUM"))

        def as_col(ap):
            return bass.AP(tensor=ap.tensor, offset=ap.offset,
                           ap=list(ap.ap) + [[0, 1]])

        # ---- one need-ordered DMA stream on sync; tiny consts on gpsimd ----
        thQ = inp.tile([128, 4, BLOC, L], f16, name="thQ")
        thK = inp.tile([128, 4, BLOC, L], f16, name="thK")
        thV = inp.tile([128, 4, BLOC, L], f16, name="thV")
        Ce_sb = consts.tile([128, FC, F], f16, name="Ce_sb")
        Se_sb = consts.tile([128, FC, F], f16, name="Se_sb")
        Ci_sb = consts.tile([128, FC, F], f16, name="Ci_sb")
        Si_sb = consts.tile([128, FC, F], f16, name="Si_sb")

        for b in range(BLOC):
            nc.sync.dma_start(out=thQ[:, :, b, :],
                              in_=Qf[b].rearrange("(c p) l -> p c l", p=128))
        for b in range(BLOC):
            nc.sync.dma_start(out=thK[:, :, b, :],
                              in_=Kf[b].rearrange("(c p) l -> p c l", p=128))
        nc.scalar.dma_start(out=Ce_sb, in_=Cef.rearrange("(a p) x -> p a x", p=128))
        nc.scalar.dma_start(out=Se_sb, in_=Sef.rearrange("(a p) x -> p a x", p=128))

        Wl_sb = consts.tile([128, 4, DH], f16, name="Wl_sb")
        Wu_sb = consts.tile([128, 4, DH], f16, name="Wu_sb")
        nc.gpsimd.dma_start(out=Wl_sb, in_=Wlf.rearrange("(c p) h -> p c h", p=128))
        nc.gpsimd.dma_start(out=Wu_sb, in_=Wuf.rearrange("(c p) h -> p c h", p=128))
        bcol = consts.tile([128, 1], f32, name="bcol")
        nc.gpsimd.dma_start(out=bcol, in_=as_col(Bcf[:]))
        nc.gpsimd.dma_start(out=Ci_sb, in_=Cif.rearrange("(a p) x -> p a x", p=128))
        nc.gpsimd.dma_start(out=Si_sb, in_=Sif.rearrange("(a p) x -> p a x", p=128))
        for b in range(BLOC):
            nc.gpsimd.dma_start(out=thV[:, :, b, :],
                                in_=Vf[b].rearrange("(c p) l -> p c l", p=128))

        # scalar act-table warm-up, off the critical path
        warm = small.tile([128, 1], f32, name="warm")
        nc.gpsimd.memset(warm, 0.0)
        nc.scalar.activation(warm, warm, AF.Exp, bias=0.0, scale=1.0)

        identh = consts.tile([128, 128], f16, name="identh")
        make_identity(nc, identh)

        # ---- projection (bias-free) + even/odd fold + transpose to rows ----
        # rows_x[p, c, r]: c in 0..3 even chunks (j' = 128c+p+1), 4..7 odd.
        rows_q = rowsp.tile([128, JC, 128], f16, name="rows_q")
        rows_k = rowsp.tile([128, JC, 128], f16, name="rows_k")
        rows_v = rowsp.tile([128, JC, 128], f16, name="rows_v")
        vsa = small.tile([DH, 4], f32, name="vsa")

        def project(th, Wsb, rows_dst, x0col, is_v):
            tp = ps_tr.tile([128, JC, 128], f16, tag="tr")
            for b in range(BLOC):
                projT = pjp.tile([DH, L], f16, tag="projT")
                for hh in range(2):
                    pj = ps_pj.tile([DH, 512], f32, tag="pj")
                    for dc in range(4):
                        nc.tensor.matmul(pj, lhsT=Wsb[:, dc, :],
                                         rhs=th[:, dc, b, hh * 512:(hh + 1) * 512],
                                         start=dc == 0, stop=dc == 3)
                    acc = vsa[:, 2 * b + hh:2 * b + hh + 1] if is_v else None
                    if hh == 0 or is_v:
                        nc.scalar.activation(projT[:, hh * 512:(hh + 1) * 512], pj,
                                             AF.Copy, bias=0.0, scale=1.0,
                                             accum_out=acc)
                    else:
                        nc.vector.tensor_copy(projT[:, hh * 512:(hh + 1) * 512], pj)
                # x[0] column for the DC-of-fold correction (per-partition col)
                nc.vector.tensor_copy(x0col[DH * b:DH * (b + 1), :], projT[:, 0:1])
                # even/odd fold along time: pe/po[j'-1] = x[j'] +- x[1024-j']
                pe = pjp.tile([DH, F], f16, tag="pe")
                po = pjp.tile([DH, F], f16, tag="po")
                rev = projT[:, 512:1024][:, ::-1]
                nc.vector.tensor_add(pe, projT[:, 1:513], rev)
                nc.vector.tensor_sub(po, projT[:, 1:513], rev)
                for c in range(FC):
                    nc.tensor.transpose(tp[:, c, DH * b:DH * (b + 1)],
                                        pe[:, c * 128:(c + 1) * 128],
                                        identh[:DH, :DH])
                    nc.tensor.transpose(tp[:, FC + c, DH * b:DH * (b + 1)],
                                        po[:, c * 128:(c + 1) * 128],
                                        identh[:DH, :DH])
            nc.scalar.activation(rows_dst, tp, AF.Copy, bias=0.0, scale=1.0)

        q0col = small.tile([128, 1], f32, name="q0col")
        k0col = small.tile([128, 1], f32, name="k0col")
        v0col = small.tile([128, 1], f32, name="v0col")
        project(thQ, Wl_sb, rows_q, q0col, False)
        project(thK, Wu_sb, rows_k, k0col, False)

        # ---- folded forward DFT: Re from even chunks/Ce, Im from odd/Se ----
        def fwd(rows_src):
            psr = ps_sp.tile([128, F], f32, tag="spec")
            psi = ps_sp.tile([128, F], f32, tag="spec")
            for c in range(FC):
                st, sp = c == 0, c == FC - 1
                nc.tensor.matmul(psr, lhsT=rows_src[:, c, :], rhs=Ce_sb[:, c, :],
                                 start=st, stop=sp)
                nc.tensor.matmul(psi, lhsT=rows_src[:, FC + c, :], rhs=Se_sb[:, c, :],
                                 start=st, stop=sp)
            return psr, psi

        QRp, QIp = fwd(rows_q)
        # stage the Q spectra in SBUF (q0 folded into Re) so the pointwise
        # products keep K's accumulators as their single PSUM operand
        QR16 = specp.tile([128, F], f16, name="QR16")
        QI16 = specp.tile([128, F], f16, name="QI16")
        nc.vector.tensor_scalar(QR16, QRp, scalar1=q0col, scalar2=None, op0=ALU.add)
        nc.scalar.activation(QI16, QIp, AF.Copy, bias=0.0, scale=1.0)

        KRp, KIp = fwd(rows_k)
        KR16 = specp.tile([128, F], f16, name="KR16")
        KI16 = specp.tile([128, F], f16, name="KI16")
        nc.vector.tensor_scalar(KR16, KRp, scalar1=k0col, scalar2=None, op0=ALU.add)
        nc.scalar.activation(KI16, KIp, AF.Copy, bias=0.0, scale=1.0)

        # ---- pointwise X = Qhat * conj(Khat), all f16 in SBUF ----
        t1 = rowbig.tile([128, F], f16, name="t1")
        t2 = rowbig.tile([128, F], f16, name="t2")
        XR = specp.tile([128, F], f16, name="XR")
        XI = specp.tile([128, F], f16, name="XI")
        nc.vector.tensor_mul(t1, QR16, KR16)
        nc.vector.tensor_mul(t2, QI16, KI16)
        nc.vector.tensor_add(XR, t1, t2)
        nc.vector.tensor_mul(t1, QI16, KR16)
        nc.vector.tensor_mul(t2, QR16, KI16)
        nc.vector.tensor_sub(XI, t1, t2)

        project(thV, Wl_sb, rows_v, v0col, True)

        # ---- chunk-transpose a [r, n*128] tile to [p, n, r] layout ----
        def to_chunks(src, nch, use_scalar=False):
            tp = ps_tr.tile([128, JC, 128], f16, tag="tr")
            for fc in range(nch):
                nc.tensor.transpose(tp[:, fc, :], src[:, fc * 128:(fc + 1) * 128],
                                    identh)
            dst = specp.tile([128, nch, 128], f16, name=f"T{src.tensor.name}")
            if use_scalar:
                nc.scalar.activation(dst, tp[:, 0:nch, :], AF.Copy,
                                     bias=0.0, scale=1.0)
            else:
                nc.vector.tensor_copy(dst, tp[:, 0:nch, :])
            return dst

        XRT = to_chunks(XR, FC)
        XIT = to_chunks(XI, FC, use_scalar=True)

        # ---- inverse DFT 1, mirror-folded: A[tau<512] even, B odd ----
        def inv_fold(RT, IT):
            Aps = ps_sp.tile([128, F], f32, tag="spec")
            Bps = ps_sp.tile([128, F], f32, tag="spec")
            for fc in range(FC):
                st, sp = fc == 0, fc == FC - 1
                nc.tensor.matmul(Aps, lhsT=RT[:, fc, :], rhs=Ci_sb[:, fc, :],
                                 start=st, stop=sp)
                # full 512-wide (junk lands in col 511; odd widths miscompute)
                nc.tensor.matmul(Bps, lhsT=IT[:, fc, :], rhs=Si_sb[:, fc, :],
                                 start=st, stop=sp)
            for fc in range(FC):
                # start=True resets col 511, replacing the junk with A[512]
                nc.tensor.matmul(Bps[:, 511:512], lhsT=RT[:, fc, :],
                                 rhs=Si_sb[:, fc, 511:512],
                                 start=fc == 0, stop=fc == FC - 1)
            return Aps, Bps

        Aps, Bps = inv_fold(XRT, XIT)
        B16 = specp.tile([128, F], f16, name="B16")
        nc.scalar.activation(B16, Bps, AF.Copy, bias=0.0, scale=1.0)
        corr16 = rowbig.tile([128, L], f16, name="corr16")
        nc.vector.tensor_copy(corr16[:, 0:1], Aps[:, 0:1])
        nc.vector.tensor_add(corr16[:, 1:512], Aps[:, 1:512], B16[:, 0:511])
        nc.vector.tensor_copy(corr16[:, 512:513], B16[:, 511:512])
        nc.vector.tensor_sub(corr16[:, 513:1024], Aps[:, 1:512][:, ::-1],
                             B16[:, 0:511][:, ::-1])

        VRp, VIp = fwd(rows_v)
        VR16 = specp.tile([128, F], f16, name="VR16")
        VI16 = specp.tile([128, F], f16, name="VI16")
        nc.vector.tensor_scalar(VR16, VRp, scalar1=v0col, scalar2=None, op0=ALU.add)
        nc.scalar.activation(VI16, VIp, AF.Copy, bias=0.0, scale=1.0)

        # ---- top-13 straight from PSUM; unnormalized masked weights ----
        vals16 = small.tile([128, 16], f16, name="vals16")
        corr2 = rowbig.tile([128, L], f16, name="corr2")
        nc.vector.max(out=vals16[:, 0:8], in_=corr16)
        nc.vector.match_replace(out=corr2, in_to_replace=vals16[:, 0:8],
                                in_values=corr16, imm_value=-60000.0)
        nc.vector.max(out=vals16[:, 8:16], in_=corr2)
        negm = small.tile([128, 1], f32, name="negm")
        nc.vector.tensor_scalar_mul(negm, vals16[:, 0:1], -1.0)
        ecorr = rowbig.tile([128, L], f16, name="ecorr")
        nc.scalar.activation(ecorr, corr16, AF.Exp, bias=negm, scale=1.0)
        em = rowbig.tile([128, L], f16, name="em")
        ssum = small.tile([128, 1], f32, name="ssum")
        nc.vector.scalar_tensor_tensor(em, in0=corr16, scalar=vals16[:, 12:13],
                                       in1=ecorr, op0=ALU.is_ge, op1=ALU.mult,
                                       accum_out=ssum)
        rs = small.tile([128, 1], f32, name="rs")
        nc.vector.reciprocal(rs, ssum)

        # DC correction column: rowsum(v') + bq
        corrcol = small.tile([128, 1], f32, name="corrcol")
        nc.vector.tensor_add(corrcol[0:DH, :], vsa[:, 0:1], vsa[:, 1:2])
        nc.vector.tensor_add(corrcol[DH:128, :], vsa[:, 2:3], vsa[:, 3:4])
        nc.vector.tensor_add(corrcol, corrcol, bcol)

        # ---- stage 2: fold em, transpose, fwd(s), Y = Vhat * conj(Shat) ----
        s0col = small.tile([128, 1], f32, name="s0col")
        nc.vector.tensor_copy(s0col, em[:, 0:1])
        sef = rowbig.tile([128, F], f16, name="sef")
        sof = rowbig.tile([128, F], f16, name="sof")
        emrev = em[:, 512:1024][:, ::-1]
        nc.vector.tensor_add(sef, em[:, 1:513], emrev)
        nc.vector.tensor_sub(sof, em[:, 1:513], emrev)

        sT = rowsp.tile([128, JC, 128], f16, name="sT")
        tp = ps_tr.tile([128, JC, 128], f16, tag="tr")
        for c in range(FC):
            nc.tensor.transpose(tp[:, c, :], sef[:, c * 128:(c + 1) * 128], identh)
            nc.tensor.transpose(tp[:, FC + c, :], sof[:, c * 128:(c + 1) * 128],
                                identh)
        nc.vector.tensor_copy(sT, tp)

        SRp, SIp = fwd(sT)
        SR16 = specp.tile([128, F], f16, name="SR16")
        SI16 = specp.tile([128, F], f16, name="SI16")
        nc.vector.tensor_scalar(SR16, SRp, scalar1=s0col, scalar2=None, op0=ALU.add)
        nc.scalar.activation(SI16, SIp, AF.Copy, bias=0.0, scale=1.0)

        u1 = rowbig.tile([128, F], f16, name="u1")
        u2 = rowbig.tile([128, F], f16, name="u2")
        YR = specp.tile([128, F], f16, name="YR")
        YI = specp.tile([128, F], f16, name="YI")
        nc.vector.tensor_mul(u1, VR16, SR16)
        nc.vector.tensor_mul(u2, VI16, SI16)
        nc.vector.tensor_add(YR, u1, u2)
        nc.vector.tensor_mul(u1, VI16, SR16)
        nc.vector.tensor_mul(u2, VR16, SI16)
        nc.vector.tensor_sub(YI, u1, u2)

        YRT = to_chunks(YR, FC)
        YIT = to_chunks(YI, FC, use_scalar=True)

        # ---- inverse DFT 2, mirror-folded, * (1/ssum) + DC column ----
        A2, B2 = inv_fold(YRT, YIT)
        B216 = specp.tile([128, F], f16, name="B216")
        nc.scalar.activation(B216, B2, AF.Copy, bias=0.0, scale=1.0)
        out16 = rowbig.tile([128, L], f16, name="out16")
        tL = rowbig.tile([128, 511], f16, name="tL")
        tH = rowbig.tile([128, 511], f16, name="tH")
        nc.vector.tensor_add(tL, A2[:, 1:512], B216[:, 0:511])
        nc.vector.tensor_sub(tH, A2[:, 1:512][:, ::-1], B216[:, 0:511][:, ::-1])
        nc.vector.tensor_scalar(out16[:, 0:1], A2[:, 0:1], scalar1=rs,
                                scalar2=corrcol, op0=ALU.mult, op1=ALU.add)
        nc.vector.tensor_scalar(out16[:, 1:512], tL, scalar1=rs,
                                scalar2=corrcol, op0=ALU.mult, op1=ALU.add)
        nc.vector.tensor_scalar(out16[:, 512:513], B216[:, 511:512], scalar1=rs,
                                scalar2=corrcol, op0=ALU.mult, op1=ALU.add)
        nc.vector.tensor_scalar(out16[:, 513:1024], tH, scalar1=rs,
                                scalar2=corrcol, op0=ALU.mult, op1=ALU.add)
        nc.sync.dma_start(out=outd[:, :], in_=out16)

    nc.compile()
    return nc


_cache = threading.Lock(), {}


def _get_nc():
    lock, store = _cache
    with lock:
        if "nc" not in store:
            store["nc"] = _build_nc()
        return store["nc"]


def _make_consts():
    j = np.arange(L, dtype=np.float64)
    fv = np.arange(1, F + 1, dtype=np.float64)
    jj = np.arange(1, F + 1, dtype=np.float64)   # folded time j' = 1..512
    Ce = np.cos(2.0 * np.pi * np.outer(jj, fv) / L)
    Ce[-1] *= 0.5                                 # j'=512 self-paired
    Se = -np.sin(2.0 * np.pi * np.outer(jj, fv) / L)
    alpha = np.full((F, 1), 2.0)
    alpha[-1, 0] = 1.0
    angi = 2.0 * np.pi * np.outer(fv, j) / L
    Ci = alpha * np.cos(angi)
    Si = alpha * -np.sin(angi)
    Ci2 = Ci[:, 0:F]
    Si2 = np.concatenate([Si[:, 1:F], Ci[:, F:F + 1]], axis=1)
    return (Ce.astype(np.float16), Se.astype(np.float16),
            Ci2.astype(np.float16), Si2.astype(np.float16))


def _make_in_maps(Q, K, V, Wq, bq):
    Q = np.ascontiguousarray(Q, np.float32)
    K = np.ascontiguousarray(K, np.float32)
    V = np.ascontiguousarray(V, np.float32)
    Wq = np.ascontiguousarray(Wq, np.float32)
    bq = np.ascontiguousarray(bq, np.float32)

    def tr16(x):
        return np.ascontiguousarray(np.swapaxes(x, 1, 2).astype(np.float16))

    Qt, Kt, Vt = tr16(Q), tr16(K), tr16(V)
    Ce, Se, Ci, Si = _make_consts()
    Wl16 = (Wq / L).astype(np.float16)
    Wu16 = Wq.astype(np.float16)
    bc = np.concatenate([bq, bq]).astype(np.float32)
    in_maps = []
    for c in range(NCORES):
        sl = slice(BLOC * c, BLOC * (c + 1))
        in_maps.append(
            {
                "Qf": Qt[sl], "Kf": Kt[sl], "Vf": Vt[sl],
                "Wlf": Wl16, "Wuf": Wu16, "Bcf": bc,
                "Cef": Ce, "Sef": Se, "Cif": Ci, "Sif": Si,
            }
        )
    return in_maps


def _assemble(outs):
    # outs[c]: [128, L] f16, rows r = 64*b + dh for batches (2c, 2c+1)
    parts = []
    for c in range(NCORES):
        r = outs[c].reshape(BLOC, DH, L)          # [b, dh, tau]
        parts.append(np.swapaxes(r, 1, 2))        # [b, tau, dh]
    compact = np.concatenate(parts, axis=0).astype(np.float32)
    return np.tile(compact, (1, 1, H))


def kernel(Q, K, V, Wq, bq):
    from concourse.bass_utils import run_bass_kernel_spmd

    nc = _get_nc()
    in_maps = _make_in_maps(Q, K, V, Wq, bq)
    res = run_bass_kernel_spmd(nc, in_maps, list(range(NCORES)))
    return _assemble([res.results[i]["out"] for i in range(NCORES)])


# revision 17
# speedup vs baseline: 1.2277x; 1.2277x over previous
"""Trainium2 Bass kernel for nn_Autocorrelation (B=16, L=1024, D=512, H=8, dh=64).

Self-contained: kernel(**inputs) -> np.ndarray [16, 1024, 512] float32.

Math restructuring vs a straight FFT port:
- Real-input spectrum f in [1, 512] only. The f=0 (DC) term is dropped: in
  stage 1 it shifts every corr lag by a per-row constant (top-k selection and
  softmax are shift-invariant); in stage 2 it is a per-row constant
  rowsum(v')*sum(s) restored via the final copy's bias column. alpha
  (2 except Nyquist) is folded into the host inverse matrices Ci/Si.
- A time-constant projection bias only feeds f=0, so q/k/v are projected
  without bq; bq reappears only in the stage-2 DC correction. 1/L rides the
  q and v projections (Wl = Wq/L).
- Even/odd folding halves the forward DFTs: with xe/xo[j'] = x[j'] +- x[L-j']
  (j'=1..512, Nyquist row halved in Ce), Re X = xe @ Ce + x[0], Im X = xo @ Se.
  The x[0] columns enter as fused scalar adds inside the pointwise complex
  products (scalar_tensor_tensor with a per-partition column).
- Softmax normalization 1/sum is per-row, and everything downstream of the
  weights is linear in them, so the kernel correlates with the UNNORMALIZED
  masked weights and multiplies 1/sum into the final output copy.
- top-13 threshold via two DVE max8 rounds + match_replace, read straight
  from corr's PSUM accumulator (no SBUF staging of corr).
- One need-ordered DMA stream on the sync queue paces the kernel:
  Q, Ce, Se, K, Ci, Si, V; output is [row=(b,dh), tau] f16, host reorders.
"""

import threading

import numpy as np

L = 1024
D = 512
DH = 64
BLOC = 2          # batches per core
B = 16
H = 8
KTOP = 13
NCORES = 8
F = 512
FC = 4            # 128-chunks in the folded/spectral dim
JC = 8


def _build_nc(cfg=None):
    from contextlib import ExitStack

    import concourse.bass as bass
    import concourse.mybir as mybir
    import concourse.tile as tile
    from concourse import bacc
    from concourse.masks import make_identity

    f32 = mybir.dt.float32
    f16 = mybir.dt.float16
    AF = mybir.ActivationFunctionType
    ALU = mybir.AluOpType

    nc = bacc.Bacc("TRN2", target_bir_lowering=False, debug=False, num_devices=NCORES)

    Qf = nc.declare_dram_parameter("Qf", [BLOC, D, L], f16, isOutput=False)
    Kf = nc.declare_dram_parameter("Kf", [BLOC, D, L], f16, isOutput=False)
    Vf = nc.declare_dram_parameter("Vf", [BLOC, D, L], f16, isOutput=False)
    Wlf = nc.declare_dram_parameter("Wlf", [D, DH], f16, isOutput=False)   # Wq/L
    Wuf = nc.declare_dram_parameter("Wuf", [D, DH], f16, isOutput=False)   # Wq
    Bcf = nc.declare_dram_parameter("Bcf", [128], f32, isOutput=False)     # tile(bq,2)
    Cef = nc.declare_dram_parameter("Cef", [F, F], f16, isOutput=False)    # folded fwd
    Sef = nc.declare_dram_parameter("Sef", [F, F], f16, isOutput=False)
    Cif = nc.declare_dram_parameter("Cif", [F, F], f16, isOutput=False)    # alpha-folded, tau 0..511
    Sif = nc.declare_dram_parameter("Sif", [F, F], f16, isOutput=False)    # [Si tau=1..511 | Ci tau=512]
    outd = nc.declare_dram_parameter("out", [128, L], f16, isOutput=True)

    with tile.TileContext(nc) as tc, ExitStack() as ctx:
        consts = ctx.enter_context(tc.tile_pool(name="consts", bufs=1))
        inp = ctx.enter_context(tc.tile_pool(name="inp", bufs=1))
        pjp = ctx.enter_context(tc.tile_pool(name="pjp", bufs=2))
        rowsp = ctx.enter_context(tc.tile_pool(name="rowsp", bufs=1))
        specp = ctx.enter_context(tc.tile_pool(name="specp", bufs=1))
        rowbig = ctx.enter_context(tc.tile_pool(name="rowbig", bufs=1))
        small = ctx.enter_context(tc.tile_pool(name="small", bufs=1))
        ps_pj = ctx.enter_context(tc.tile_pool(name="ps_pj", bufs=2, space="PSUM"))
        ps_tr = ctx.enter_context(tc.tile_pool(name="ps_tr", bufs=2, space="PSUM"))
        ps_sp = ctx.enter_context(tc.tile_pool(name="ps_sp", bufs=4, space="PSUM"))

        def as_col(ap):
            return bass.AP(tensor=ap.tensor, offset=ap.offset,
                           ap=list(ap.ap) + [[0, 1]])

        # ---- one need-ordered DMA stream on sync; tiny consts on gpsimd ----
        thQ = inp.tile([128, 4, BLOC, L], f16, name="thQ")
        thK = inp.tile([128, 4, BLOC, L], f16, name="thK")
        thV = inp.tile([128, 4, BLOC, L], f16, name="thV")
        Ce_sb = consts.tile([128, FC, F], f16, name="Ce_sb")
        Se_sb = consts.tile([128, FC, F], f16, name="Se_sb")
        Ci_sb = consts.tile([128, FC, F], f16, name="Ci_sb")
        Si_sb = consts.tile([128, FC, F], f16, name="Si_sb")

        for b in range(BLOC):
            nc.sync.dma_start(out=thQ[:, :, b, :],
                              in_=Qf[b].rearrange("(c p) l -> p c l", p=128))
        for b in range(BLOC):
            nc.sync.dma_start(out=thK[:, :, b, :],
                              in_=Kf[b].rearrange("(c p) l -> p c l", p=128))
        for b in range(BLOC):
            nc.sync.dma_start(out=thV[:, :, b, :],
                              in_=Vf[b].rearrange("(c p) l -> p c l", p=128))
        nc.scalar.dma_start(out=Ce_sb, in_=Cef.rearrange("(a p) x -> p a x", p=128))
        nc.scalar.dma_start(out=Se_sb, in_=Sef.rearrange("(a p) x -> p a x", p=128))

        Wl_sb = consts.tile([128, 4, DH], f16, name="Wl_sb")
        Wu_sb = consts.tile([128, 4, DH], f16, name="Wu_sb")
        nc.gpsimd.dma_start(out=Wl_sb, in_=Wlf.rearrange("(c p) h -> p c h", p=128))
        nc.gpsimd.dma_start(out=Wu_sb, in_=Wuf.rearrange("(c p) h -> p c h", p=128))
        bcol = consts.tile([128, 1], f32, name="bcol")
        nc.gpsimd.dma_start(out=bcol, in_=as_col(Bcf[:]))
        nc.gpsimd.dma_start(out=Ci_sb, in_=Cif.rearrange("(a p) x -> p a x", p=128))
        nc.gpsimd.dma_start(out=Si_sb, in_=Sif.rearrange("(a p) x -> p a x", p=128))

        # scalar act-table warm-up, off the critical path
        warm = small.tile([128, 1], f32, name="warm")
        nc.gpsimd.memset(warm, 0.0)
        nc.scalar.activation(warm, warm, AF.Exp, bias=0.0, scale=1.0)

        identh = consts.tile([128, 128], f16, name="identh")
        make_identity(nc, identh)

        # ---- projection (bias-free) + even/odd fold + transpose to rows ----
        # rows_x[p, c, r]: c in 0..3 even chunks (j' = 128c+p+1), 4..7 odd.
        rows_q = rowsp.tile([128, JC, 128], f16, name="rows_q")
        rows_k = rowsp.tile([128, JC, 128], f16, name="rows_k")
        rows_v = rowsp.tile([128, JC, 128], f16, name="rows_v")
        vsa = small.tile([DH, 4], f32, name="vsa")

        def project(th, Wsb, rows_dst, x0col, is_v):
            tp = ps_tr.tile([128, JC, 128], f16, tag="tr")
            for b in range(BLOC):
                projT = pjp.tile([DH, L], f16, tag="projT")
                for hh in range(2):
                    pj = ps_pj.tile([DH, 512], f32, tag="pj")
                    for dc in range(4):
                        nc.tensor.matmul(pj, lhsT=Wsb[:, dc, :],
                                         rhs=th[:, dc, b, hh * 512:(hh + 1) * 512],
                                         start=dc == 0, stop=dc == 3)
                    acc = vsa[:, 2 * b + hh:2 * b + hh + 1] if is_v else None
                    nc.scalar.activation(projT[:, hh * 512:(hh + 1) * 512], pj,
                                         AF.Copy, bias=0.0, scale=1.0,
                                         accum_out=acc)
                # x[0] column for the DC-of-fold correction (per-partition col)
                nc.gpsimd.tensor_copy(x0col[DH * b:DH * (b + 1), :], projT[:, 0:1])
                # even/odd fold along time: pe/po[j'-1] = x[j'] +- x[1024-j']
                pe = pjp.tile([DH, F], f16, tag="pe")
                po = pjp.tile([DH, F], f16, tag="po")
                rev = projT[:, 512:1024][:, ::-1]
                nc.vector.tensor_add(pe, projT[:, 1:513], rev)
                nc.gpsimd.tensor_sub(po, projT[:, 1:513], rev)
                for c in range(FC):
                    nc.tensor.transpose(tp[:, c, DH * b:DH * (b + 1)],
                                        pe[:, c * 128:(c + 1) * 128],
                                        identh[:DH, :DH])
                    nc.tensor.transpose(tp[:, FC + c, DH * b:DH * (b + 1)],
                                        po[:, c * 128:(c + 1) * 128],
                                        identh[:DH, :DH])
            nc.scalar.activation(rows_dst, tp, AF.Copy, bias=0.0, scale=1.0)

        q0col = small.tile([128, 1], f32, name="q0col")
        k0col = small.tile([128, 1], f32, name="k0col")
        v0col = small.tile([128, 1], f32, name="v0col")
        project(thQ, Wl_sb, rows_q, q0col, False)
        project(thK, Wu_sb, rows_k, k0col, False)

        # ---- folded forward DFT: Re from even chunks/Ce, Im from odd/Se ----
        def fwd(rows_src):
            psr = ps_sp.tile([128, F], f32, tag="spec")
            psi = ps_sp.tile([128, F], f32, tag="spec")
            for c in range(FC):
                st, sp = c == 0, c == FC - 1
                nc.tensor.matmul(psr, lhsT=rows_src[:, c, :], rhs=Ce_sb[:, c, :],
                                 start=st, stop=sp)
                nc.tensor.matmul(psi, lhsT=rows_src[:, FC + c, :], rhs=Se_sb[:, c, :],
                                 start=st, stop=sp)
            return psr, psi

        QRp, QIp = fwd(rows_q)
        # stage the Q spectra in SBUF (q0 folded into Re) so the pointwise
        # products keep K's accumulators as their single PSUM operand
        QR16 = specp.tile([128, F], f16, name="QR16")
        QI16 = specp.tile([128, F], f16, name="QI16")
        nc.vector.tensor_scalar(QR16, QRp, scalar1=q0col, scalar2=None, op0=ALU.add)
        nc.scalar.activation(QI16, QIp, AF.Copy, bias=0.0, scale=1.0)

        KRp, KIp = fwd(rows_k)
        KR16 = specp.tile([128, F], f16, name="KR16")
        KI16 = specp.tile([128, F], f16, name="KI16")
        nc.vector.tensor_scalar(KR16, KRp, scalar1=k0col, scalar2=None, op0=ALU.add)
        nc.scalar.activation(KI16, KIp, AF.Copy, bias=0.0, scale=1.0)

        # ---- pointwise X = Qhat * conj(Khat), all f16 in SBUF ----
        t1 = rowbig.tile([128, F], f16, name="t1")
        t2 = rowbig.tile([128, F], f16, name="t2")
        XR = specp.tile([128, F], f16, name="XR")
        XI = specp.tile([128, F], f16, name="XI")
        nc.vector.tensor_mul(t1, QR16, KR16)
        nc.vector.tensor_mul(t2, QI16, KI16)
        nc.vector.tensor_add(XR, t1, t2)
        nc.vector.tensor_mul(t1, QI16, KR16)
        nc.vector.tensor_mul(t2, QR16, KI16)
        nc.vector.tensor_sub(XI, t1, t2)

        project(thV, Wl_sb, rows_v, v0col, True)

        # ---- chunk-transpose a [r, n*128] tile to [p, n, r] layout ----
        def to_chunks(src, nch, use_scalar=False):
            tp = ps_tr.tile([128, JC, 128], f16, tag="tr")
            for fc in range(nch):
                nc.tensor.transpose(tp[:, fc, :], src[:, fc * 128:(fc + 1) * 128],
                                    identh)
            dst = specp.tile([128, nch, 128], f16, name=f"T{src.tensor.name}")
            if use_scalar:
                nc.scalar.activation(dst, tp[:, 0:nch, :], AF.Copy,
                                     bias=0.0, scale=1.0)
            else:
                nc.vector.tensor_copy(dst, tp[:, 0:nch, :])
            return dst

        XRT = to_chunks(XR, FC)
        XIT = to_chunks(XI, FC, use_scalar=True)

        # ---- inverse DFT 1, mirror-folded: A[tau<512] even, B odd ----
        def inv_fold(RT, IT):
            Aps = ps_sp.tile([128, F], f32, tag="spec")
            Bps = ps_sp.tile([128, F], f32, tag="spec")
            for fc in range(FC):
                st, sp = fc == 0, fc == FC - 1
                nc.tensor.matmul(Aps, lhsT=RT[:, fc, :], rhs=Ci_sb[:, fc, :],
                                 start=st, stop=sp)
                # full 512-wide (junk lands in col 511; odd widths miscompute)
                nc.tensor.matmul(Bps, lhsT=IT[:, fc, :], rhs=Si_sb[:, fc, :],
                                 start=st, stop=sp)
            for fc in range(FC):
                # start=True resets col 511, replacing the junk with A[512]
                nc.tensor.matmul(Bps[:, 511:512], lhsT=RT[:, fc, :],
                                 rhs=Si_sb[:, fc, 511:512],
                                 start=fc == 0, stop=fc == FC - 1)
            return Aps, Bps

        Aps, Bps = inv_fold(XRT, XIT)
        B16 = specp.tile([128, F], f16, name="B16")
        nc.scalar.activation(B16, Bps, AF.Copy, bias=0.0, scale=1.0)
        corr16 = rowbig.tile([128, L], f16, name="corr16")
        nc.vector.tensor_copy(corr16[:, 0:1], Aps[:, 0:1])
        nc.vector.tensor_add(corr16[:, 1:512], Aps[:, 1:512], B16[:, 0:511])
        nc.vector.tensor_copy(corr16[:, 512:513], B16[:, 511:512])
        nc.vector.tensor_sub(corr16[:, 513:1024], Aps[:, 1:512][:, ::-1],
                             B16[:, 0:511][:, ::-1])

        VRp, VIp = fwd(rows_v)
        VR16 = specp.tile([128, F], f16, name="VR16")
        VI16 = specp.tile([128, F], f16, name="VI16")
        nc.vector.tensor_scalar(VR16, VRp, scalar1=v0col, scalar2=None, op0=ALU.add)
        nc.scalar.activation(VI16, VIp, AF.Copy, bias=0.0, scale=1.0)

        # ---- top-13 straight from PSUM; unnormalized masked weights ----
        vals16 = small.tile([128, 16], f16, name="vals16")
        corr2 = rowbig.tile([128, L], f16, name="corr2")
        nc.vector.max(out=vals16[:, 0:8], in_=corr16)
        nc.vector.match_replace(out=corr2, in_to_replace=vals16[:, 0:8],
                                in_values=corr16, imm_value=-60000.0)
        nc.vector.max(out=vals16[:, 8:16], in_=corr2)
        negm = small.tile([128, 1], f32, name="negm")
        nc.vector.tensor_scalar_mul(negm, vals16[:, 0:1], -1.0)
        ecorr = rowbig.tile([128, L], f16, name="ecorr")
        nc.scalar.activation(ecorr, corr16, AF.Exp, bias=negm, scale=1.0)
        em = rowbig.tile([128, L], f16, name="em")
        ssum = small.tile([128, 1], f32, name="ssum")
        nc.vector.scalar_tensor_tensor(em, in0=corr16, scalar=vals16[:, 12:13],
                                       in1=ecorr, op0=ALU.is_ge, op1=ALU.mult,
                                       accum_out=ssum)
        rs = small.tile([128, 1], f32, name="rs")
        nc.vector.reciprocal(rs, ssum)

        # DC correction column: rowsum(v') + bq
        corrcol = small.tile([128, 1], f32, name="corrcol")
        nc.vector.tensor_add(corrcol[0:DH, :], vsa[:, 0:1], vsa[:, 1:2])
        nc.vector.tensor_add(corrcol[DH:128, :], vsa[:, 2:3], vsa[:, 3:4])
        nc.vector.tensor_add(corrcol, corrcol, bcol)

        # ---- stage 2: fold em, transpose, fwd(s), Y = Vhat * conj(Shat) ----
        s0col = small.tile([128, 1], f32, name="s0col")
        nc.vector.tensor_copy(s0col, em[:, 0:1])
        sef = rowbig.tile([128, F], f16, name="sef")
        sof = rowbig.tile([128, F], f16, name="sof")
        emrev = em[:, 512:1024][:, ::-1]
        nc.vector.tensor_add(sef, em[:, 1:513], emrev)
        nc.vector.tensor_sub(sof, em[:, 1:513], emrev)

        sT = rowsp.tile([128, JC, 128], f16, name="sT")
        tp = ps_tr.tile([128, JC, 128], f16, tag="tr")
        for c in range(FC):
            nc.tensor.transpose(tp[:, c, :], sef[:, c * 128:(c + 1) * 128], identh)
            nc.tensor.transpose(tp[:, FC + c, :], sof[:, c * 128:(c + 1) * 128],
                                identh)
        nc.vector.tensor_copy(sT, tp)

        SRp, SIp = fwd(sT)
        SR16 = specp.tile([128, F], f16, name="SR16")
        SI16 = specp.tile([128, F], f16, name="SI16")
        nc.vector.tensor_scalar(SR16, SRp, scalar1=s0col, scalar2=None, op0=ALU.add)
        nc.scalar.activation(SI16, SIp, AF.Copy, bias=0.0, scale=1.0)

        u1 = rowbig.tile([128, F], f16, name="u1")
        u2 = rowbig.tile([128, F], f16, name="u2")
        YR = specp.tile([128, F], f16, name="YR")
        YI = specp.tile([128, F], f16, name="YI")
        nc.vector.tensor_mul(u1, VR16, SR16)
        nc.vector.tensor_mul(u2, VI16, SI16)
        nc.vector.tensor_add(YR, u1, u2)
        nc.vector.tensor_mul(u1, VI16, SR16)
        nc.vector.tensor_mul(u2, VR16, SI16)
        nc.vector.tensor_sub(YI, u1, u2)

        YRT = to_chunks(YR, FC)
        YIT = to_chunks(YI, FC, use_scalar=True)

        # ---- inverse DFT 2, mirror-folded, * (1/ssum) + DC column ----
        A2, B2 = inv_fold(YRT, YIT)
        B216 = specp.tile([128, F], f16, name="B216")
        nc.scalar.activation(B216, B2, AF.Copy, bias=0.0, scale=1.0)
        out16 = rowbig.tile([128, L], f16, name="out16")
        tL = rowbig.tile([128, 511], f16, name="tL")
        tH = rowbig.tile([128, 511], f16, name="tH")
        nc.vector.tensor_add(tL, A2[:, 1:512], B216[:, 0:511])
        nc.vector.tensor_sub(tH, A2[:, 1:512][:, ::-1], B216[:, 0:511][:, ::-1])
        nc.vector.tensor_scalar(out16[:, 0:1], A2[:, 0:1], scalar1=rs,
                                scalar2=corrcol, op0=ALU.mult, op1=ALU.add)
        nc.vector.tensor_scalar(out16[:, 1:512], tL, scalar1=rs,
                                scalar2=corrcol, op0=ALU.mult, op1=ALU.add)
        nc.vector.tensor_scalar(out16[:, 512:513], B216[:, 511:512], scalar1=rs,
                                scalar2=corrcol, op0=ALU.mult, op1=ALU.add)
        nc.vector.tensor_scalar(out16[:, 513:1024], tH, scalar1=rs,
                                scalar2=corrcol, op0=ALU.mult, op1=ALU.add)
        nc.sync.dma_start(out=outd[:, :], in_=out16)

    nc.compile()
    return nc


_cache = threading.Lock(), {}


def _get_nc():
    lock, store = _cache
    with lock:
        if "nc" not in store:
            store["nc"] = _build_nc()
        return store["nc"]


def _make_consts():
    j = np.arange(L, dtype=np.float64)
    fv = np.arange(1, F + 1, dtype=np.float64)
    jj = np.arange(1, F + 1, dtype=np.float64)   # folded time j' = 1..512
    Ce = np.cos(2.0 * np.pi * np.outer(jj, fv) / L)
    Ce[-1] *= 0.5                                 # j'=512 self-paired
    Se = -np.sin(2.0 * np.pi * np.outer(jj, fv) / L)
    alpha = np.full((F, 1), 2.0)
    alpha[-1, 0] = 1.0
    angi = 2.0 * np.pi * np.outer(fv, j) / L
    Ci = alpha * np.cos(angi)
    Si = alpha * -np.sin(angi)
    Ci2 = Ci[:, 0:F]
    Si2 = np.concatenate([Si[:, 1:F], Ci[:, F:F + 1]], axis=1)
    return (Ce.astype(np.float16), Se.astype(np.float16),
            Ci2.astype(np.float16), Si2.astype(np.float16))


def _make_in_maps(Q, K, V, Wq, bq):
    Q = np.ascontiguousarray(Q, np.float32)
    K = np.ascontiguousarray(K, np.float32)
    V = np.ascontiguousarray(V, np.float32)
    Wq = np.ascontiguousarray(Wq, np.float32)
    bq = np.ascontiguousarray(bq, np.float32)

    def tr16(x):
        return np.ascontiguousarray(np.swapaxes(x, 1, 2).astype(np.float16))

    Qt, Kt, Vt = tr16(Q), tr16(K), tr16(V)
    Ce, Se, Ci, Si = _make_consts()
    Wl16 = (Wq / L).astype(np.float16)
    Wu16 = Wq.astype(np.float16)
    bc = np.concatenate([bq, bq]).astype(np.float32)
    in_maps = []
    for c in range(NCORES):
        sl = slice(BLOC * c, BLOC * (c + 1))
        in_maps.append(
            {
                "Qf": Qt[sl], "Kf": Kt[sl], "Vf": Vt[sl],
                "Wlf": Wl16, "Wuf": Wu16, "Bcf": bc,
                "Cef": Ce, "Sef": Se, "Cif": Ci, "Sif": Si,
            }
        )
    return in_maps


def _assemble(outs):
    # outs[c]: [128, L] f16, rows r = 64*b + dh for batches (2c, 2c+1)
    parts = []
    for c in range(NCORES):
        r = outs[c].reshape(BLOC, DH, L)          # [b, dh, tau]
        parts.append(np.swapaxes(r, 1, 2))        # [b, tau, dh]
    compact = np.concatenate(parts, axis=0).astype(np.float32)
    return np.tile(compact, (1, 1, H))


def kernel(Q, K, V, Wq, bq):
    from concourse.bass_utils import run_bass_kernel_spmd

    nc = _get_nc()
    in_maps = _make_in_maps(Q, K, V, Wq, bq)
    res = run_bass_kernel_spmd(nc, in_maps, list(range(NCORES)))
    return _assemble([res.results[i]["out"] for i in range(NCORES)])


# revision 18
# speedup vs baseline: 1.2863x; 1.0477x over previous
"""Trainium2 Bass kernel for nn_Autocorrelation (B=16, L=1024, D=512, H=8, dh=64).

Self-contained: kernel(**inputs) -> np.ndarray [16, 1024, 512] float32.

Math restructuring vs a straight FFT port:
- Real-input spectrum f in [1, 512] only. The f=0 (DC) term is dropped: in
  stage 1 it shifts every corr lag by a per-row constant (top-k selection and
  softmax are shift-invariant); in stage 2 it is a per-row constant
  rowsum(v')*sum(s) restored via the final copy's bias column. alpha
  (2 except Nyquist) is folded into the host inverse matrices Ci/Si.
- A time-constant projection bias only feeds f=0, so q/k/v are projected
  without bq; bq reappears only in the stage-2 DC correction. 1/L rides the
  q and v projections (Wl = Wq/L).
- Even/odd folding halves the forward DFTs: with xe/xo[j'] = x[j'] +- x[L-j']
  (j'=1..512, Nyquist row halved in Ce), Re X = xe @ Ce + x[0], Im X = xo @ Se.
  The x[0] columns enter as fused scalar adds inside the pointwise complex
  products (scalar_tensor_tensor with a per-partition column).
- Softmax normalization 1/sum is per-row, and everything downstream of the
  weights is linear in them, so the kernel correlates with the UNNORMALIZED
  masked weights and multiplies 1/sum into the final output copy.
- top-13 threshold via two DVE max8 rounds + match_replace, read straight
  from corr's PSUM accumulator (no SBUF staging of corr).
- One need-ordered DMA stream on the sync queue paces the kernel:
  Q, Ce, Se, K, Ci, Si, V; output is [row=(b,dh), tau] f16, host reorders.
"""

import threading

import numpy as np

L = 1024
D = 512
DH = 64
BLOC = 2          # batches per core
B = 16
H = 8
KTOP = 13
NCORES = 8
F = 512
FC = 4            # 128-chunks in the folded/spectral dim
JC = 8


def _build_nc(cfg=None):
    from contextlib import ExitStack

    import concourse.bass as bass
    import concourse.mybir as mybir
    import concourse.tile as tile
    from concourse import bacc
    from concourse.masks import make_identity

    f32 = mybir.dt.float32
    f16 = mybir.dt.float16
    AF = mybir.ActivationFunctionType
    ALU = mybir.AluOpType

    nc = bacc.Bacc("TRN2", target_bir_lowering=False, debug=False, num_devices=NCORES)

    Qf = nc.declare_dram_parameter("Qf", [BLOC, D, L], f16, isOutput=False)
    Kf = nc.declare_dram_parameter("Kf", [BLOC, D, L], f16, isOutput=False)
    Vf = nc.declare_dram_parameter("Vf", [BLOC, D, L], f16, isOutput=False)
    Wlf = nc.declare_dram_parameter("Wlf", [D, DH], f16, isOutput=False)   # Wq/L
    Wuf = nc.declare_dram_parameter("Wuf", [D, DH], f16, isOutput=False)   # Wq
    Bcf = nc.declare_dram_parameter("Bcf", [128], f32, isOutput=False)     # tile(bq,2)
    Cef = nc.declare_dram_parameter("Cef", [F, F], f16, isOutput=False)    # folded fwd
    Sef = nc.declare_dram_parameter("Sef", [F, F], f16, isOutput=False)
    Cif = nc.declare_dram_parameter("Cif", [F, F], f16, isOutput=False)    # alpha-folded, tau 0..511
    Sif = nc.declare_dram_parameter("Sif", [F, F], f16, isOutput=False)    # [Si tau=1..511 | Ci tau=512]
    outd = nc.declare_dram_parameter("out", [128, L], f16, isOutput=True)

    with tile.TileContext(nc) as tc, ExitStack() as ctx:
        consts = ctx.enter_context(tc.tile_pool(name="consts", bufs=1))
        inp = ctx.enter_context(tc.tile_pool(name="inp", bufs=1))
        pjp = ctx.enter_context(tc.tile_pool(name="pjp", bufs=2))
        rowsp = ctx.enter_context(tc.tile_pool(name="rowsp", bufs=1))
        specp = ctx.enter_context(tc.tile_pool(name="specp", bufs=1))
        rowbig = ctx.enter_context(tc.tile_pool(name="rowbig", bufs=1))
        small = ctx.enter_context(tc.tile_pool(name="small", bufs=1))
        ps_pj = ctx.enter_context(tc.tile_pool(name="ps_pj", bufs=2, space="PSUM"))
        ps_tr = ctx.enter_context(tc.tile_pool(name="ps_tr", bufs=2, space="PSUM"))
        ps_sp = ctx.enter_context(tc.tile_pool(name="ps_sp", bufs=4, space="PSUM"))

        def as_col(ap):
            return bass.AP(tensor=ap.tensor, offset=ap.offset,
                           ap=list(ap.ap) + [[0, 1]])

        # ---- one need-ordered DMA stream on sync; tiny consts on gpsimd ----
        thQ = inp.tile([128, 4, BLOC, L], f16, name="thQ")
        thK = inp.tile([128, 4, BLOC, L], f16, name="thK")
        thV = inp.tile([128, 4, BLOC, L], f16, name="thV")
        Ce_sb = consts.tile([128, FC, F], f16, name="Ce_sb")
        Se_sb = consts.tile([128, FC, F], f16, name="Se_sb")
        Ci_sb = consts.tile([128, FC, F], f16, name="Ci_sb")
        Si_sb = consts.tile([128, FC, F], f16, name="Si_sb")

        for b in range(BLOC):
            nc.sync.dma_start(out=thQ[:, :, b, :],
                              in_=Qf[b].rearrange("(c p) l -> p c l", p=128))
        for b in range(BLOC):
            nc.sync.dma_start(out=thK[:, :, b, :],
                              in_=Kf[b].rearrange("(c p) l -> p c l", p=128))
        nc.sync.dma_start(out=Ci_sb, in_=Cif.rearrange("(a p) x -> p a x", p=128))
        nc.sync.dma_start(out=Si_sb, in_=Sif.rearrange("(a p) x -> p a x", p=128))
        for b in range(BLOC):
            nc.sync.dma_start(out=thV[:, :, b, :],
                              in_=Vf[b].rearrange("(c p) l -> p c l", p=128))
        nc.scalar.dma_start(out=Ce_sb, in_=Cef.rearrange("(a p) x -> p a x", p=128))
        nc.scalar.dma_start(out=Se_sb, in_=Sef.rearrange("(a p) x -> p a x", p=128))

        Wl_sb = consts.tile([128, 4, DH], f16, name="Wl_sb")
        Wu_sb = consts.tile([128, 4, DH], f16, name="Wu_sb")
        nc.gpsimd.dma_start(out=Wl_sb, in_=Wlf.rearrange("(c p) h -> p c h", p=128))
        nc.gpsimd.dma_start(out=Wu_sb, in_=Wuf.rearrange("(c p) h -> p c h", p=128))
        bcol = consts.tile([128, 1], f32, name="bcol")
        nc.gpsimd.dma_start(out=bcol, in_=as_col(Bcf[:]))

        # scalar act-table warm-up, off the critical path
        warm = small.tile([128, 1], f32, name="warm")
        nc.gpsimd.memset(warm, 0.0)
        nc.scalar.activation(warm, warm, AF.Exp, bias=0.0, scale=1.0)

        identh = consts.tile([128, 128], f16, name="identh")
        make_identity(nc, identh)

        # ---- projection (bias-free) + even/odd fold + transpose to rows ----
        # rows_x[p, c, r]: c in 0..3 even chunks (j' = 128c+p+1), 4..7 odd.
        rows_q = rowsp.tile([128, JC, 128], f16, name="rows_q")
        rows_k = rowsp.tile([128, JC, 128], f16, name="rows_k")
        rows_v = rowsp.tile([128, JC, 128], f16, name="rows_v")
        vsa = small.tile([DH, 4], f32, name="vsa")

        def project(th, Wsb, rows_dst, x0col, is_v):
            tp = ps_tr.tile([128, JC, 128], f16, tag="tr")
            for b in range(BLOC):
                projT = pjp.tile([DH, L], f16, tag="projT")
                for hh in range(2):
                    pj = ps_pj.tile([DH, 512], f32, tag="pj")
                    for dc in range(4):
                        nc.tensor.matmul(pj, lhsT=Wsb[:, dc, :],
                                         rhs=th[:, dc, b, hh * 512:(hh + 1) * 512],
                                         start=dc == 0, stop=dc == 3)
                    acc = vsa[:, 2 * b + hh:2 * b + hh + 1] if is_v else None
                    nc.scalar.activation(projT[:, hh * 512:(hh + 1) * 512], pj,
                                         AF.Copy, bias=0.0, scale=1.0,
                                         accum_out=acc)
                # x[0] column for the DC-of-fold correction (per-partition col)
                nc.gpsimd.tensor_copy(x0col[DH * b:DH * (b + 1), :], projT[:, 0:1])
                # even/odd fold along time: pe/po[j'-1] = x[j'] +- x[1024-j']
                pe = pjp.tile([DH, F], f16, tag="pe")
                po = pjp.tile([DH, F], f16, tag="po")
                rev = projT[:, 512:1024][:, ::-1]
                nc.vector.tensor_add(pe, projT[:, 1:513], rev)
                nc.gpsimd.tensor_sub(po, projT[:, 1:513], rev)
                for c in range(FC):
                    nc.tensor.transpose(tp[:, c, DH * b:DH * (b + 1)],
                                        pe[:, c * 128:(c + 1) * 128],
                                        identh[:DH, :DH])
                    nc.tensor.transpose(tp[:, FC + c, DH * b:DH * (b + 1)],
                                        po[:, c * 128:(c + 1) * 128],
                                        identh[:DH, :DH])
            nc.scalar.activation(rows_dst, tp, AF.Copy, bias=0.0, scale=1.0)

        q0col = small.tile([128, 1], f32, name="q0col")
        k0col = small.tile([128, 1], f32, name="k0col")
        v0col = small.tile([128, 1], f32, name="v0col")
        project(thQ, Wl_sb, rows_q, q0col, False)
        project(thK, Wu_sb, rows_k, k0col, False)

        # ---- folded forward DFT: Re from even chunks/Ce, Im from odd/Se ----
        def fwd(rows_src):
            psr = ps_sp.tile([128, F], f32, tag="spec")
            psi = ps_sp.tile([128, F], f32, tag="spec")
            for c in range(FC):
                st, sp = c == 0, c == FC - 1
                nc.tensor.matmul(psr, lhsT=rows_src[:, c, :], rhs=Ce_sb[:, c, :],
                                 start=st, stop=sp)
                nc.tensor.matmul(psi, lhsT=rows_src[:, FC + c, :], rhs=Se_sb[:, c, :],
                                 start=st, stop=sp)
            return psr, psi

        QRp, QIp = fwd(rows_q)
        # stage the Q spectra in SBUF (q0 folded into Re) so the pointwise
        # products keep K's accumulators as their single PSUM operand
        QR16 = specp.tile([128, F], f16, name="QR16")
        QI16 = specp.tile([128, F], f16, name="QI16")
        nc.vector.tensor_scalar(QR16, QRp, scalar1=q0col, scalar2=None, op0=ALU.add)
        nc.scalar.activation(QI16, QIp, AF.Copy, bias=0.0, scale=1.0)

        KRp, KIp = fwd(rows_k)
        KR16 = specp.tile([128, F], f16, name="KR16")
        KI16 = specp.tile([128, F], f16, name="KI16")
        nc.vector.tensor_scalar(KR16, KRp, scalar1=k0col, scalar2=None, op0=ALU.add)
        nc.scalar.activation(KI16, KIp, AF.Copy, bias=0.0, scale=1.0)

        # ---- pointwise X = Qhat * conj(Khat), all f16 in SBUF ----
        t1 = rowbig.tile([128, F], f16, name="t1")
        t2 = rowbig.tile([128, F], f16, name="t2")
        XR = specp.tile([128, F], f16, name="XR")
        XI = specp.tile([128, F], f16, name="XI")
        nc.vector.tensor_mul(t1, QR16, KR16)
        nc.vector.tensor_mul(t2, QI16, KI16)
        nc.vector.tensor_add(XR, t1, t2)
        nc.vector.tensor_mul(t1, QI16, KR16)
        nc.vector.tensor_mul(t2, QR16, KI16)
        nc.vector.tensor_sub(XI, t1, t2)

        project(thV, Wl_sb, rows_v, v0col, True)

        # ---- chunk-transpose a [r, n*128] tile to [p, n, r] layout ----
        def to_chunks(src, nch, use_scalar=False):
            tp = ps_tr.tile([128, JC, 128], f16, tag="tr")
            for fc in range(nch):
                nc.tensor.transpose(tp[:, fc, :], src[:, fc * 128:(fc + 1) * 128],
                                    identh)
            dst = specp.tile([128, nch, 128], f16, name=f"T{src.tensor.name}")
            if use_scalar:
                nc.scalar.activation(dst, tp[:, 0:nch, :], AF.Copy,
                                     bias=0.0, scale=1.0)
            else:
                nc.vector.tensor_copy(dst, tp[:, 0:nch, :])
            return dst

        XRT = to_chunks(XR, FC)
        XIT = to_chunks(XI, FC, use_scalar=True)

        # ---- inverse DFT 1, mirror-folded: A[tau<512] even, B odd ----
        def inv_fold(RT, IT):
            Aps = ps_sp.tile([128, F], f32, tag="spec")
            Bps = ps_sp.tile([128, F], f32, tag="spec")
            for fc in range(FC):
                st, sp = fc == 0, fc == FC - 1
                nc.tensor.matmul(Aps, lhsT=RT[:, fc, :], rhs=Ci_sb[:, fc, :],
                                 start=st, stop=sp)
                # full 512-wide (junk lands in col 511; odd widths miscompute)
                nc.tensor.matmul(Bps, lhsT=IT[:, fc, :], rhs=Si_sb[:, fc, :],
                                 start=st, stop=sp)
            for fc in range(FC):
                # start=True resets col 511, replacing the junk with A[512]
                nc.tensor.matmul(Bps[:, 511:512], lhsT=RT[:, fc, :],
                                 rhs=Si_sb[:, fc, 511:512],
                                 start=fc == 0, stop=fc == FC - 1)
            return Aps, Bps

        Aps, Bps = inv_fold(XRT, XIT)
        B16 = specp.tile([128, F], f16, name="B16")
        nc.scalar.activation(B16, Bps, AF.Copy, bias=0.0, scale=1.0)
        corr16 = rowbig.tile([128, L], f16, name="corr16")
        nc.vector.tensor_copy(corr16[:, 0:1], Aps[:, 0:1])
        nc.vector.tensor_add(corr16[:, 1:512], Aps[:, 1:512], B16[:, 0:511])
        nc.vector.tensor_copy(corr16[:, 512:513], B16[:, 511:512])
        nc.vector.tensor_sub(corr16[:, 513:1024], Aps[:, 1:512][:, ::-1],
                             B16[:, 0:511][:, ::-1])

        VRp, VIp = fwd(rows_v)
        VR16 = specp.tile([128, F], f16, name="VR16")
        VI16 = specp.tile([128, F], f16, name="VI16")
        nc.vector.tensor_scalar(VR16, VRp, scalar1=v0col, scalar2=None, op0=ALU.add)
        nc.scalar.activation(VI16, VIp, AF.Copy, bias=0.0, scale=1.0)

        # ---- top-13 straight from PSUM; unnormalized masked weights ----
        vals16 = small.tile([128, 16], f16, name="vals16")
        corr2 = rowbig.tile([128, L], f16, name="corr2")
        nc.vector.max(out=vals16[:, 0:8], in_=corr16)
        nc.vector.match_replace(out=corr2, in_to_replace=vals16[:, 0:8],
                                in_values=corr16, imm_value=-60000.0)
        nc.vector.max(out=vals16[:, 8:16], in_=corr2)
        negm = small.tile([128, 1], f32, name="negm")
        nc.vector.tensor_scalar_mul(negm, vals16[:, 0:1], -1.0)
        ecorr = rowbig.tile([128, L], f16, name="ecorr")
        nc.scalar.activation(ecorr, corr16, AF.Exp, bias=negm, scale=1.0)
        em = rowbig.tile([128, L], f16, name="em")
        ssum = small.tile([128, 1], f32, name="ssum")
        nc.vector.scalar_tensor_tensor(em, in0=corr16, scalar=vals16[:, 12:13],
                                       in1=ecorr, op0=ALU.is_ge, op1=ALU.mult,
                                       accum_out=ssum)
        rs = small.tile([128, 1], f32, name="rs")
        nc.vector.reciprocal(rs, ssum)

        # DC correction column: rowsum(v') + bq
        corrcol = small.tile([128, 1], f32, name="corrcol")
        nc.vector.tensor_add(corrcol[0:DH, :], vsa[:, 0:1], vsa[:, 1:2])
        nc.vector.tensor_add(corrcol[DH:128, :], vsa[:, 2:3], vsa[:, 3:4])
        nc.vector.tensor_add(corrcol, corrcol, bcol)

        # ---- stage 2: fold em, transpose, fwd(s), Y = Vhat * conj(Shat) ----
        s0col = small.tile([128, 1], f32, name="s0col")
        nc.vector.tensor_copy(s0col, em[:, 0:1])
        sef = rowbig.tile([128, F], f16, name="sef")
        sof = rowbig.tile([128, F], f16, name="sof")
        emrev = em[:, 512:1024][:, ::-1]
        nc.vector.tensor_add(sef, em[:, 1:513], emrev)
        nc.vector.tensor_sub(sof, em[:, 1:513], emrev)

        sT = rowsp.tile([128, JC, 128], f16, name="sT")
        tp = ps_tr.tile([128, JC, 128], f16, tag="tr")
        for c in range(FC):
            nc.tensor.transpose(tp[:, c, :], sef[:, c * 128:(c + 1) * 128], identh)
            nc.tensor.transpose(tp[:, FC + c, :], sof[:, c * 128:(c + 1) * 128],
                                identh)
        nc.vector.tensor_copy(sT, tp)

        SRp, SIp = fwd(sT)
        SR16 = specp.tile([128, F], f16, name="SR16")
        SI16 = specp.tile([128, F], f16, name="SI16")
        nc.vector.tensor_scalar(SR16, SRp, scalar1=s0col, scalar2=None, op0=ALU.add)
        nc.scalar.activation(SI16, SIp, AF.Copy, bias=0.0, scale=1.0)

        u1 = rowbig.tile([128, F], f16, name="u1")
        u2 = rowbig.tile([128, F], f16, name="u2")
        YR = specp.tile([128, F], f16, name="YR")
        YI = specp.tile([128, F], f16, name="YI")
        nc.vector.tensor_mul(u1, VR16, SR16)
        nc.vector.tensor_mul(u2, VI16, SI16)
        nc.vector.tensor_add(YR, u1, u2)
        nc.vector.tensor_mul(u1, VI16, SR16)
        nc.vector.tensor_mul(u2, VR16, SI16)
        nc.vector.tensor_sub(YI, u1, u2)

        YRT = to_chunks(YR, FC)
        YIT = to_chunks(YI, FC, use_scalar=True)

        # ---- inverse DFT 2, mirror-folded, * (1/ssum) + DC column ----
        A2, B2 = inv_fold(YRT, YIT)
        B216 = specp.tile([128, F], f16, name="B216")
        nc.scalar.activation(B216, B2, AF.Copy, bias=0.0, scale=1.0)
        out16 = rowbig.tile([128, L], f16, name="out16")
        tL = rowbig.tile([128, 511], f16, name="tL")
        tH = rowbig.tile([128, 511], f16, name="tH")
        nc.vector.tensor_add(tL, A2[:, 1:512], B216[:, 0:511])
        nc.vector.tensor_sub(tH, A2[:, 1:512][:, ::-1], B216[:, 0:511][:, ::-1])
        nc.vector.tensor_scalar(out16[:, 0:1], A2[:, 0:1], scalar1=rs,
                                scalar2=corrcol, op0=ALU.mult, op1=ALU.add)
        nc.vector.tensor_scalar(out16[:, 1:512], tL, scalar1=rs,
                                scalar2=corrcol, op0=ALU.mult, op1=ALU.add)
        nc.vector.tensor_scalar(out16[:, 512:513], B216[:, 511:512], scalar1=rs,
                                scalar2=corrcol, op0=ALU.mult, op1=ALU.add)
        nc.vector.tensor_scalar(out16[:, 513:1024], tH, scalar1=rs,
                                scalar2=corrcol, op0=ALU.mult, op1=ALU.add)
        nc.sync.dma_start(out=outd[:, :], in_=out16)

    nc.compile()
    return nc


_cache = threading.Lock(), {}


def _get_nc():
    lock, store = _cache
    with lock:
        if "nc" not in store:
            store["nc"] = _build_nc()
        return store["nc"]


def _make_consts():
    j = np.arange(L, dtype=np.float64)
    fv = np.arange(1, F + 1, dtype=np.float64)
    jj = np.arange(1, F + 1, dtype=np.float64)   # folded time j' = 1..512
    Ce = np.cos(2.0 * np.pi * np.outer(jj, fv) / L)
    Ce[-1] *= 0.5                                 # j'=512 self-paired
    Se = -np.sin(2.0 * np.pi * np.outer(jj, fv) / L)
    alpha = np.full((F, 1), 2.0)
    alpha[-1, 0] = 1.0
    angi = 2.0 * np.pi * np.outer(fv, j) / L
    Ci = alpha * np.cos(angi)
    Si = alpha * -np.sin(angi)
    Ci2 = Ci[:, 0:F]
    Si2 = np.concatenate([Si[:, 1:F], Ci[:, F:F + 1]], axis=1)
    return (Ce.astype(np.float16), Se.astype(np.float16),
            Ci2.astype(np.float16), Si2.astype(np.float16))


def _make_in_maps(Q, K, V, Wq, bq):
    Q = np.ascontiguousarray(Q, np.float32)
    K = np.ascontiguousarray(K, np.float32)
    V = np.ascontiguousarray(V, np.float32)
    Wq = np.ascontiguousarray(Wq, np.float32)
    bq = np.ascontiguousarray(bq, np.float32)

    def tr16(x):
        return np.ascontiguousarray(np.swapaxes(x, 1, 2).astype(np.float16))

    Qt, Kt, Vt = tr16(Q), tr16(K), tr16(V)
    Ce, Se, Ci, Si = _make_consts()
    Wl16 = (Wq / L).astype(np.float16)
    Wu16 = Wq.astype(np.float16)
    bc = np.concatenate([bq, bq]).astype(np.float32)
    in_maps = []
    for c in range(NCORES):
        sl = slice(BLOC * c, BLOC * (c + 1))
        in_maps.append(
            {
                "Qf": Qt[sl], "Kf": Kt[sl], "Vf": Vt[sl],
                "Wlf": Wl16, "Wuf": Wu16, "Bcf": bc,
                "Cef": Ce, "Sef": Se, "Cif": Ci, "Sif": Si,
            }
        )
    return in_maps


def _assemble(outs):
    # outs[c]: [128, L] f16, rows r = 64*b + dh for batches (2c, 2c+1)
    parts = []
    for c in range(NCORES):
        r = outs[c].reshape(BLOC, DH, L)          # [b, dh, tau]
        parts.append(np.swapaxes(r, 1, 2))        # [b, tau, dh]
    compact = np.concatenate(parts, axis=0).astype(np.float32)
    return np.tile(compact, (1, 1, H))


def kernel(Q, K, V, Wq, bq):
    from concourse.bass_utils import run_bass_kernel_spmd

    nc = _get_nc()
    in_maps = _make_in_maps(Q, K, V, Wq, bq)
    res = run_bass_kernel_spmd(nc, in_maps, list(range(NCORES)))
    return _assemble([res.results[i]["out"] for i in range(NCORES)])
